# revision 23
# baseline (speedup 1.0000x reference)
"""Trainium2 Bass kernel for nn_Net_21852793602541 (gnn_message_passing).

The reference net's output depends only on a tiny dependency cone of the
message-passing graph: the final hidden layer reads a snapshot of neuron
activations, so only neurons feeding neuron 255 through channels whose
source was already processed matter.  For the fixed graph that is a 3-conv
chain (x -> n0 -> n172 -> n215), one 784->200 FC block, a 200->10 FC and
log_softmax.  The cone is recomputed at runtime from the src/tgt inputs.

Per-core mapping (data-parallel over batch, 16 images/core on 8 cores):
  * each 5x5 conv = 5 accumulating K=32 matmuls: stationary T_dy [32,28]
    (x-banded weights), moving = a [64, B*32] activation tile read at 5
    different y-column offsets.  The tile holds TWO copies of the
    activation (partitions 0-31 / 32-63), written in parallel by ACT and
    DVE straight from conv PSUM, so the dy matmuls split 3+2 across two
    PE row groups and run concurrently.
  * fc1 runs transposed: stationary = fc1-weight slices, moving = the
    (yg,x)-packed activation b-columns, accumulating hidden as [h, b]
    directly in PSUM; relu+bias is a per-partition ACT op, feeding fc2 as
    the stationary operand to produce logits in [b, cls] orientation with
    fc2_b folded in via a constant-1 contraction row.
  * log_softmax without max-subtraction (logits are O(1)); exp-with-
    accumulation, ln, and one DVE tensor_scalar subtract.
  * all activations (relu/copy/exp/ln) are steered to the single
    natural_log_exp_and_others table so exactly one ACT_TABLE_LOAD runs,
    during the input-DMA wait.
"""

import numpy as np

import concourse.bass as bass
import concourse.tile as tile
from concourse import bacc, mybir
from concourse.bass_utils import run_bass_kernel_spmd

# The axon NTFF profile hook normally lives in antenv.axon_hooks, which this
# image lacks.  Shim it from the boot module's ctypes implementation so
# BASS_TRACE=1 profiling works; degrade silently if unavailable.
try:
    import antenv.axon_hooks  # noqa: F401
except ImportError:
    try:
        import sys as _sys
        import types as _types

        from trn_agent_boot.trn_boot import _ntff_profile_via_ctypes

        _hook = _ntff_profile_via_ctypes('/opt/axon/libaxon_pjrt.so')
        _mod = _types.ModuleType('antenv.axon_hooks')
        _mod.get_axon_ntff_profile_hook = lambda: _hook
        _mod.set_axon_ntff_profile_hook = lambda h: None
        _sys.modules['antenv.axon_hooks'] = _mod
    except Exception:
        pass

# Steer every activation we use (exp/ln/relu/identity/copy) to the one
# act-func table that contains them all, so the compiler's table-load pass
# emits a single ACT_TABLE_LOAD (id semantics are preserved: the dict keeps
# the full act_info.json entry order, only the non-target sets are thinned).
import concourse.hw_specs as _hw_specs
import concourse.bacc as _bacc_mod

_orig_get_act_tables = _hw_specs.get_activation_tables


def _patched_act_tables(arch):
    tabs = _orig_get_act_tables(arch)
    target = 'natural_log_exp_and_others'
    if target not in tabs:
        return tabs
    t6 = tabs[target]
    return {name: (set(s) if name == target else set(s) - t6)
            for name, s in tabs.items()}


_hw_specs.get_activation_tables = _patched_act_tables
_bacc_mod.get_activation_tables = _patched_act_tables

F32 = mybir.dt.float32
F16 = mybir.dt.float16
AF = mybir.ActivationFunctionType
ALU = mybir.AluOpType
N_NEURONS = 256
N_CORES = 8
B_TOTAL = 128
B = B_TOTAL // N_CORES  # 16 images per core
HW = 28
FC_HID = 200
N_CLS = 10

LAST_RESULT = None  # BassKernelResults of the most recent run (for profiling)


# ---------------------------------------------------------------- schedule
def _schedule(src, tgt):
    n = N_NEURONS
    in_lists = [src[np.where(tgt == i)[0]].astype(np.int64).tolist() for i in range(n)]
    waves = []
    processed = np.zeros(n, bool)
    frontier = [0]
    while True:
        waves.append(list(frontier))
        processed[frontier] = True
        if processed[n - 1]:
            break
        nxt = set()
        for v in frontier:
            for m in tgt[src == v]:
                if not processed[m]:
                    nxt.add(int(m))
        frontier = sorted(nxt)
        assert frontier, "last neuron unreachable"
    return in_lists, waves


def _cone(src, tgt):
    """Returns (steps, fc_live).

    steps: ordered list of (node, [(srckey, channel), ...]) where srckey is
      'x' for the image input or an int neuron id computed in an earlier step.
    fc_live: [(channel_of_255, src_node), ...] live channels of the readout.
    """
    n = N_NEURONS
    in_lists, waves = _schedule(src, tgt)
    wave_of = {}
    for wi, w in enumerate(waves):
        for v in w:
            if v not in wave_of:
                wave_of[v] = wi
    BIG = 1 << 30
    w255 = wave_of[n - 1]
    fc_live = [(c, int(s)) for c, s in enumerate(in_lists[n - 1])
               if wave_of.get(int(s), BIG) < w255]

    live = {}
    stack = [s for _, s in fc_live]
    seen = set()
    while stack:
        v = stack.pop()
        if v in seen:
            continue
        seen.add(v)
        if v == 0:
            live[0] = [('x', 0)]
            continue
        chans = [(int(s), c) for c, s in enumerate(in_lists[v])
                 if wave_of.get(int(s), BIG) < wave_of[v]]
        assert chans, f"cone node {v} has no live channels"
        live[v] = [(s, c) for s, c in chans]
        stack += [s for s, _ in chans]

    steps = sorted(live.items(), key=lambda kv: wave_of[kv[0]])
    return steps, fc_live


# ---------------------------------------------------------- host-side packing
def _tband(w, dy):
    """w [5,5], one dy -> [32, 28] x-banded matrix: T[r, m] = w[dy, r-m+2].

    Contraction rows r are input x (image W columns); rows 28..31 multiply
    the zeroed x-padding rows of the activation tile."""
    T = np.zeros((32, HW), np.float32)
    for m in range(HW):
        for dx in range(5):
            r = m + dx - 2
            if 0 <= r < 32:
                T[r, m] = w[dy, dx]
    return T


def _xbase(xb):
    """xb [B,28,28] -> [64, B*32] fp16 (two copies): X[j, b*32+i+2] = xb[b,i,j]."""
    X = np.zeros((32, B, 32), np.float32)
    X[:HW, :, 2:30] = xb.transpose(2, 0, 1)
    X = X.reshape(32, B * 32).astype(np.float16)
    return np.concatenate([X, X], axis=0)


class _Layout:
    def __init__(self):
        self.n = 0

    def alloc(self, w):
        c0 = self.n
        self.n += w
        return c0


def _pack(steps, fc_live, conv_w, conv_b, fc1_w, fc1_b, fc2_w, fc2_b):
    """Builds consts (f32 [128, n32]), mainh (fp16 [32, n16]), f1w (fp16)."""
    slots = {}
    lay32 = _Layout()
    lay16 = _Layout()
    for v, chans in steps:
        for j, _ in enumerate(chans):
            for dy in range(5):
                slots[('t', v, j, dy)] = lay16.alloc(HW)
        slots[('cb', v)] = lay32.alloc(1)
    slots['fc1bA'] = lay32.alloc(1)
    slots['fc1bB'] = lay32.alloc(1)
    toep_cols = lay16.n
    slots['xs'] = lay16.alloc(B * 32)

    C = np.zeros((128, lay32.n), np.float32)
    TH = np.zeros((32, toep_cols), np.float16)
    for v, chans in steps:
        for j, (skey, ch) in enumerate(chans):
            for dy in range(5):
                c0 = slots[('t', v, j, dy)]
                TH[:, c0:c0 + HW] = _tband(conv_w[v, 0, ch], dy)
        C[:HW, slots[('cb', v)]] = conv_b[v]
    TH = np.concatenate([TH, TH], axis=0)  # both PE row groups
    C[:128, slots['fc1bA']] = fc1_b[:128]
    C[:FC_HID - 128, slots['fc1bB']] = fc1_b[128:]

    n_fc = len(fc_live)
    f1p = np.zeros((128, 1400 * n_fc + 2 * N_CLS), np.float16)
    for k, (c, s) in enumerate(fc_live):
        blk = fc1_w[:, c * 784:(c + 1) * 784].reshape(FC_HID, HW, HW)  # [h, i, j]
        arr = blk.reshape(FC_HID, 4, 7, HW).transpose(1, 3, 2, 0)  # [yg, j, ysub, h]
        f1p[:, k * 1400:(k + 1) * 1400] = np.pad(
            arr, ((0, 0), (0, 4), (0, 0), (0, 0))).reshape(128, 1400)
    w2t = fc2_w.T  # [200, 10]
    w0 = 1400 * n_fc
    slots['fc2wA'] = w0
    slots['fc2wB'] = w0 + N_CLS
    f1p[:, w0:w0 + N_CLS] = w2t[:128]
    f1p[:FC_HID - 128, w0 + N_CLS:w0 + 2 * N_CLS] = w2t[128:]
    f1p[FC_HID - 128, w0 + N_CLS:w0 + 2 * N_CLS] = fc2_b  # 1s-row partner
    return C, TH, f1p, slots


# ---------------------------------------------------------- device program
def _build(steps, fc_live, ncols32, ncols16, nfc):
    nc = bacc.Bacc("TRN2", target_bir_lowering=False)
    consts_d = nc.dram_tensor("consts", [128, ncols32], F32, kind="ExternalInput")
    mainh_d = nc.dram_tensor("mainh", [64, ncols16], F16, kind="ExternalInput")
    f1wc = 1400 * nfc + 2 * N_CLS
    f1w_d = nc.dram_tensor("f1w", [128, f1wc], F16, kind="ExternalInput")
    out_d = nc.dram_tensor("out", [B, N_CLS], F32, kind="ExternalOutput")

    feeds_conv = set()
    for v, chans in steps:
        for skey, _ in chans:
            if skey != 'x':
                feeds_conv.add(skey)
    fc_srcs = [s for _, s in fc_live]
    SL = _SLOTS
    HB = FC_HID - 128  # 72

    with tile.TileContext(nc) as tc:
        with (
            tc.tile_pool(name="persist", bufs=1) as pool,
            tc.tile_pool(name="cpsum", bufs=2, space="PSUM") as cpp,
            tc.tile_pool(name="fpsum", bufs=1, space="PSUM") as fpp,
        ):
            consts = pool.tile([128, ncols32], F32, tag="consts")
            mainh = pool.tile([64, ncols16], F16, tag="mainh")
            f1w = pool.tile([128, f1wc], F16, tag="f1w")

            # activation tiles (two row-group copies each), zeroed early
            acts = {}
            for v in sorted(feeds_conv):
                a = pool.tile([64, B * 32], F16, name=f"act_{v}", tag=f"act_{v}")
                nc.vector.memset(a[:], 0.0)
                acts[v] = a
            fcstacks = {}
            for sv in set(fc_srcs):
                t = pool.tile([128, 112], F16, name=f"fcst_{sv}", tag=f"fcst_{sv}")
                nc.vector.memset(t[:], 0.0)
                fcstacks[sv] = t
            h12 = pool.tile([128, 32], F16, tag="h12")
            # fc2 bias row: row HB=72 must read 1.0; memset a 32-aligned
            # block (64:80) — rows 64..71 are overwritten by the fc1-B relu,
            # rows 73..79 are never read (fc2 contraction stops at row 72)
            nc.vector.memset(h12[64:80, 16:32], 1.0)

            # single act-table load, hoisted into the DMA wait
            swu = pool.tile([1, 2], F32, tag="swu")
            nc.vector.memset(swu[:], 1.0)
            nc.scalar.activation(swu[:, 0:1], swu[:, 0:1], AF.Exp)

            # input DMAs: critical tensors first, one mainh row group per
            # HWDGE ring so each conv row-group chain gates on its own DMA;
            # the tiny consts tensor rides the SWDGE ring off to the side
            nc.sync.dma_start(mainh[0:32, :], mainh_d[0:32, :])
            nc.scalar.dma_start(mainh[32:64, :], mainh_d[32:64, :])
            nc.gpsimd.dma_start(consts[:], consts_d[:])
            fh = f1wc // 2
            nc.sync.dma_start(f1w[:, :fh], f1w_d[:, :fh])
            nc.scalar.dma_start(f1w[:, fh:], f1w_d[:, fh:])

            # early consts touches: give each relu engine its own direct
            # wait on the consts DMA so the per-conv ACT/DVE write pairs
            # need only a Tensor wait later (and so run in parallel)
            tch = pool.tile([1, 2], F32, tag="tch")
            nc.vector.tensor_copy(tch[:, 0:1], consts[0:1, 0:1])
            nc.scalar.activation(tch[:, 1:2], consts[0:1, 0:1], AF.Copy)

            xs0 = SL['xs']

            def src_view(skey):
                t = mainh[:, xs0:xs0 + B * 32] if skey == 'x' else acts[skey][:]
                return t.rearrange("p (b q) -> p b q", q=32)

            # --- conv chain: per channel, each PE row group computes all 5
            # dy matmuls for its own y-half into its own PSUM tile, so the
            # two groups run concurrently with disjoint PSUM targets
            YH = HW // 2  # 14
            for v, chans in steps:
                cb0 = SL[('cb', v)]
                bias = consts[:HW, cb0:cb0 + 1]
                nch = len(chans)
                ph = [cpp.tile([HW, B * YH], F32, tag=f"convps{h}",
                               name=f"ps{v}_{h}") for h in range(2)]
                # interleave the two row groups pairwise so they stream
                # concurrently; h1 runs dy 2,3,4 first (those windows need
                # only the [16:30) act columns, so the next conv's h1 chain
                # starts after a single producer op)
                mm = 0
                nmm = 10 * nch
                for j, (skey, ch) in enumerate(chans):
                    av = src_view(skey)
                    for d0, d1 in zip((0, 1, 2, 3, 4), (2, 3, 4, 0, 1)):
                        for h, g, dy in ((0, 0, d0), (1, 32, d1)):
                            t0 = SL[('t', v, j, dy)]
                            y0 = dy + h * YH
                            nc.tensor.matmul(
                                ph[h][:], mainh[g:g + 32, t0:t0 + HW],
                                av[g:g + 32, :, y0:y0 + YH],
                                start=(mm < 2 and j == 0),
                                stop=(mm >= nmm - 2 and j == nch - 1))
                            mm += 1
                pv = [p[:].rearrange("p (b y) -> p b y", y=YH) for p in ph]

                if v in acts:
                    # relu+bias into both act-tile copies, cross-paired so
                    # the two engines' first ops read different PSUM tiles
                    # (overlapping reads of one tile serialize engines)
                    av = acts[v][:].rearrange("p (b q) -> p b q", q=32)
                    nc.vector.tensor_scalar(av[32:32 + HW, :, 2 + YH:2 + HW],
                                            pv[1], bias, 0.0, ALU.add, ALU.max)
                    nc.scalar.activation(av[0:HW, :, 2:2 + YH], pv[0],
                                         AF.Relu, bias=bias, scale=1.0)
                    nc.scalar.activation(av[0:HW, :, 2 + YH:2 + HW], pv[1],
                                         AF.Relu, bias=bias, scale=1.0)
                    nc.vector.tensor_scalar(av[32:32 + HW, :, 2:2 + YH], pv[0],
                                            bias, 0.0, ALU.add, ALU.max)
                if v in fcstacks:
                    # each PSUM tile's first reader waits on Tensor directly
                    # (ACT reads pv0 first, DVE pv1 first); the second
                    # readers chain behind them in parallel pairs
                    fst = fcstacks[v]
                    fv = fst[:].rearrange("p (b s) -> p b s", s=7)
                    qs = []
                    for g in range(4):
                        qs.append((fv[g * 32:g * 32 + HW, :, :],
                                   pv[g // 2][:, :, 7 * (g % 2):7 * (g % 2) + 7]))
                    nc.scalar.activation(qs[0][0], qs[0][1], AF.Relu,
                                         bias=bias, scale=1.0)
                    nc.vector.tensor_scalar(qs[2][0], qs[2][1], bias, 0.0,
                                            ALU.add, ALU.max)
                    nc.vector.tensor_scalar(qs[1][0], qs[1][1], bias, 0.0,
                                            ALU.add, ALU.max)
                    nc.scalar.activation(qs[3][0], qs[3][1], AF.Relu,
                                         bias=bias, scale=1.0)

            # --- fc1 transposed: hidden accumulates as [h, b] in PSUM ---
            p1a = fpp.tile([128, B], F32, tag="p1a")
            p1b = fpp.tile([HB, B], F32, tag="p1b")
            for k in range(nfc):
                fst = fcstacks[fc_live[k][1]]
                fv = fst[:].rearrange("p (b s) -> p b s", s=7)
                for sj in range(7):
                    i = k * 7 + sj
                    w0 = i * 200
                    nc.tensor.matmul(p1a[:], f1w[:, w0:w0 + 128], fv[:, :, sj],
                                     start=(i == 0), stop=(i == 7 * nfc - 1))
                    nc.tensor.matmul(p1b[:], f1w[:, w0 + 128:w0 + 200], fv[:, :, sj],
                                     start=(i == 0), stop=(i == 7 * nfc - 1))
            nc.scalar.activation(h12[:, 0:B], p1a[:], AF.Relu,
                                 bias=consts[:128, SL['fc1bA']:SL['fc1bA'] + 1],
                                 scale=1.0)
            nc.vector.tensor_scalar(h12[0:HB, 16:16 + B], p1b[:],
                                    consts[:HB, SL['fc1bB']:SL['fc1bB'] + 1],
                                    0.0, ALU.add, ALU.max)

            # --- fc2 into [b, cls] orientation, bias via the 1s-row ---
            ps2 = fpp.tile([B, N_CLS], F32, tag="l2")
            nc.tensor.matmul(ps2[:], h12[:, 0:B],
                             f1w[:, SL['fc2wA']:SL['fc2wA'] + N_CLS],
                             start=True, stop=False)
            nc.tensor.matmul(ps2[:], h12[0:HB + 1, 16:16 + B],
                             f1w[0:HB + 1, SL['fc2wB']:SL['fc2wB'] + N_CLS],
                             start=False, stop=True)

            # --- log_softmax: x - ln(sum(exp(x))); logits are O(1) so the
            # max-subtraction is unnecessary for fp32/table accuracy.  The
            # class-sum runs on DVE (cheaper than ACT's accumulator read) ---
            ex = pool.tile([B, N_CLS], F32, tag="ex")
            nc.scalar.activation(ex[:], ps2[:], AF.Exp, bias=0.0, scale=1.0)
            sm = pool.tile([B, 1], F32, tag="sm")
            nc.vector.reduce_sum(sm[:], ex[:], axis=mybir.AxisListType.X)
            lse = pool.tile([B, 1], F32, tag="lse")
            nc.scalar.activation(lse[:], sm[:], AF.Ln, bias=0.0, scale=1.0)
            res = pool.tile([B, N_CLS], F32, tag="res")
            nc.vector.tensor_scalar_sub(res[:], ps2[:], lse[:])
            nc.sync.dma_start(out_d[:], res[:])
    nc.compile()
    return nc


_SLOTS = None
_PROG_CACHE = {}


def kernel(x, src, tgt, conv_w, conv_b, fc1_w, fc1_b, fc2_w, fc2_b):
    global _SLOTS, LAST_RESULT
    x = np.asarray(x, np.float32)
    src = np.asarray(src, np.int32)
    tgt = np.asarray(tgt, np.int32)
    conv_w = np.asarray(conv_w, np.float32)
    conv_b = np.asarray(conv_b, np.float32)
    fc1_w = np.asarray(fc1_w, np.float32)
    fc1_b = np.asarray(fc1_b, np.float32)
    fc2_w = np.asarray(fc2_w, np.float32)
    fc2_b = np.asarray(fc2_b, np.float32)

    steps, fc_live = _cone(src, tgt)
    C, TH, f1p, slots = _pack(steps, fc_live, conv_w, conv_b,
                              fc1_w, fc1_b, fc2_w, fc2_b)
    _SLOTS = slots
    ncols16 = TH.shape[1] + B * 32

    key = (tuple((v, tuple(ch)) for v, ch in steps), tuple(fc_live),
           C.shape[1], ncols16)
    if key not in _PROG_CACHE:
        _PROG_CACHE[key] = _build(steps, fc_live, C.shape[1], ncols16,
                                  len(fc_live))
    nc = _PROG_CACHE[key]

    xs = x[:, 0]  # [128, 28, 28]
    in_maps = []
    for c in range(N_CORES):
        mainh = np.concatenate([TH, _xbase(xs[c * B:(c + 1) * B])], axis=1)
        in_maps.append({"consts": C, "mainh": mainh, "f1w": f1p})

    LAST_RESULT = run_bass_kernel_spmd(nc, in_maps, list(range(N_CORES)))
    out = np.concatenate([r["out"] for r in LAST_RESULT.results], axis=0)
    return out.astype(np.float32)
